# revision 1
# baseline (speedup 1.0000x reference)
"""TTFS (time-to-first-spike) encoder kernel for Trainium2, 8 NeuronCores.

Math: the reference runs, per element, the fp32 recurrence
    mem_k = fl(fl(mem_{k-1} * d) + fl(cur * (1-d))),   d = fl(exp(-0.5f))
and emits a one-hot over time at the first k with mem_k >= 1.0 (then masks all
later spikes).  mem_k is monotone in cur (composition of monotone rounded
ops), so "first crossing at step k" is exactly a threshold test on cur:
    spike at out[t] iff THETA[t+1] <= cur < THETA[t]      (THETA[0] = +inf)
where THETA[k] = min fp32 c with mem_k(c) >= 1.0, found by binary search over
the fp32 bit space with a bit-exact host simulation of the recurrence.  The
fp32 recurrence converges by step 32: THETA[32] == THETA[33] == ... ==
THETA[64], so out[:, t, :] == 0 for all t >= 32 for EVERY input, and the
device only computes/writes slabs t = 0..31 (the host zero-fills the rest).

Device work per core (batch-sharded 2048/8 = 256 rows):
    cur = x * sensitivity (PE broadcasts sensitivity across partitions)
    s_k = [cur >= THETA[k]]    (tensor_scalar is_ge, fp32 -> bf16)
    out[t] = s_{t+1} - s_t     (tensor_tensor subtract on {0,1} bf16, exact)
Output slabs are bf16 holding exact 0.0/1.0; host casts to fp32.
"""

import numpy as np

from concourse import bacc, mybir
from concourse import bass
from concourse import tile
from concourse.bass_utils import run_bass_kernel_spmd

# THETA[k] for k = 1..32 as fp32 bit patterns (see module docstring).
_THETA_BITS = [
    0x4022A7D7, 0x3FCA7E37, 0x3FA4C386, 0x3F9408C5,
    0x3F8B724C, 0x3F86B4E7, 0x3F83FC52, 0x3F82635E,
    0x3F81701C, 0x3F80DE49, 0x3F808677, 0x3F80516D,
    0x3F803157, 0x3F801DE8, 0x3F801222, 0x3F800B00,
    0x3F8006AB, 0x3F80040B, 0x3F800274, 0x3F80017D,
    0x3F8000E7, 0x3F80008C, 0x3F800055, 0x3F800034,
    0x3F80001F, 0x3F800013, 0x3F80000C, 0x3F800007,
    0x3F800005, 0x3F800002, 0x3F800002, 0x3F800001,
]
THETAS = np.array(_THETA_BITS, dtype=np.uint32).view(np.float32)

N_CORES = 8
B, T, N = 2048, 64, 1024
BS = B // N_CORES          # 256 batch rows per core
P = 128                    # SBUF partitions
TS = 32                    # device-computed time slabs (rest are zero)
TC = 16                    # timesteps per DMA chunk

F32 = mybir.dt.float32
BF16 = mybir.dt.bfloat16


def _build():
    nc = bacc.Bacc("TRN2", target_bir_lowering=False, debug=False)
    x_d = nc.dram_tensor("x", [BS, N], F32, kind="ExternalInput")
    sens_d = nc.dram_tensor("sens", [1, N], F32, kind="ExternalInput")
    out_d = nc.dram_tensor("out", [BS, TS, N], BF16, kind="ExternalOutput")

    with tile.TileContext(nc) as tc:
        with (
            tc.tile_pool(name="const", bufs=1) as cpool,
            tc.tile_pool(name="psum", bufs=2, space="PSUM") as ppool,
            tc.tile_pool(name="work", bufs=2) as wpool,
            tc.tile_pool(name="s", bufs=4) as spool,
            tc.tile_pool(name="slab", bufs=2) as slabpool,
        ):
            ones = cpool.tile([1, P], F32)
            nc.vector.memset(ones[:], 1.0)
            sens_sb = cpool.tile([1, N], F32)
            nc.sync.dma_start(sens_sb[:], sens_d[:, :])
            sens_bc = cpool.tile([P, N], F32)
            for half in range(2):
                ps = ppool.tile([P, 512], F32)
                nc.tensor.matmul(
                    ps[:], ones[:], sens_sb[:, half * 512:(half + 1) * 512],
                    start=True, stop=True,
                )
                nc.scalar.copy(sens_bc[:, half * 512:(half + 1) * 512], ps[:])

            for h in range(2):
                xt = wpool.tile([P, N], F32, tag="xt")
                nc.sync.dma_start(xt[:], x_d[h * P:(h + 1) * P, :])
                cur = wpool.tile([P, N], F32, tag="cur")
                nc.vector.tensor_tensor(cur[:], xt[:], sens_bc[:],
                                        mybir.AluOpType.mult)
                s_prev = None
                for tchunk in range(TS // TC):
                    slab = slabpool.tile([P, TC * N], BF16, tag="slab")
                    for tt in range(TC):
                        t = tchunk * TC + tt
                        s = spool.tile([P, N], BF16, tag="s")
                        nc.vector.tensor_scalar(
                            s[:], cur[:], float(THETAS[t]), None,
                            mybir.AluOpType.is_ge,
                        )
                        dst = slab[:, tt * N:(tt + 1) * N]
                        if t == 0:
                            nc.scalar.copy(dst, s[:])
                        else:
                            nc.vector.tensor_tensor(dst, s[:], s_prev[:],
                                                    mybir.AluOpType.subtract)
                        s_prev = s
                    nc.sync.dma_start(
                        out_d[h * P:(h + 1) * P,
                              tchunk * TC:(tchunk + 1) * TC, :],
                        slab[:],
                    )
    nc.compile()
    return nc


_NC = None


def _get_nc():
    global _NC
    if _NC is None:
        _NC = _build()
    return _NC


def kernel(x, sensitivity):
    x = np.ascontiguousarray(np.asarray(x, dtype=np.float32))
    sens = np.ascontiguousarray(np.asarray(sensitivity, dtype=np.float32))
    sens = sens.reshape(1, N)
    nc = _get_nc()
    in_maps = [
        {"x": x[c * BS:(c + 1) * BS], "sens": sens} for c in range(N_CORES)
    ]
    res = run_bass_kernel_spmd(nc, in_maps, list(range(N_CORES)))
    dev = np.concatenate(
        [np.asarray(r["out"]) for r in res.results], axis=0
    )  # [B, TS, N] bf16, exact 0/1
    out = np.zeros((B, T, N), dtype=np.float32)
    out[:, :TS, :] = dev.astype(np.float32)
    return out
